# revision 6
# baseline (speedup 1.0000x reference)
"""Trainium2 Bass kernel for the ActionableRGM problem.

Math: the reference computes, per (b, l):
    T[b,l] = S @ R(theta[b,l]) @ S^-1,   theta[b,l,m] = x[b,l] . om[m]
and scans z_{l} = T[b,l] @ z_{l-1} from z0, emitting every z_l.

Because all T share the same block-rotation structure conjugated by the same
S, the product telescopes:
    z_l = S @ R(sum_{t<=l} theta[b,t]) @ S^-1 z0
so with w = S^-1 z0, phi = cumsum(x) . om (cumulative angles):
    out[b,l,i] = sum_m cos(phi)[m] * B1[m,i] + sin(phi)[m] * B2[m,i] + w0*S[i,0]
    B1[m,:] = w[2m+1] * S[:,2m+1] + w[2m+2] * S[:,2m+2]
    B2[m,:] = w[2m+1] * S[:,2m+2] - w[2m+2] * S[:,2m+1]

The sequential scan disappears; every position is independent. We shard the
B*L = 2048 positions across 8 cores (256 each). On-device per core:
    v = phi/2pi (+0.25 for the cos lane) : two small matmuls    (TensorE)
    k = (v + MAGIC) - MAGIC  (round-to-nearest-int in f32)      (VectorE)
    u = v - k in [-1/2, 1/2]                                    (VectorE)
    sin/cos = Sin(u * 2pi) over the concatenated [128,512] lane (ScalarE)
    out = cos^T @ B1 + sin^T @ B2 + bias                        (TensorE+VectorE)
"""

import math

import numpy as np

P = 128  # frequencies (M)
D = 257  # state dim (2M+1)
B, L = 4, 512
N_CORES = 8
NPOS = (B * L) // N_CORES  # positions per core = 256

TWO_PI = 2.0 * math.pi
MAGIC = 1.5 * 2.0 ** 23  # adding this rounds |v|<2^22 to nearest integer


def _build_nc():
    import concourse.mybir as mybir
    from concourse import bacc
    from concourse.tile import TileContext

    f32 = mybir.dt.float32
    ALU = mybir.AluOpType
    nc = bacc.Bacc()

    # xct rows: [Xc_x ; Xc_y ; ones]
    xct = nc.declare_dram_parameter("xct", [3, NPOS], f32, isOutput=False)
    # omn_c rows: [om_x/2pi ; om_y/2pi ; 0.25], omn_s is the first two rows
    omn = nc.declare_dram_parameter("omn", [3, P], f32, isOutput=False)
    b1 = nc.declare_dram_parameter("b1", [P, D], f32, isOutput=False)
    b2 = nc.declare_dram_parameter("b2", [P, D], f32, isOutput=False)
    bias = nc.declare_dram_parameter("bias", [P, D], f32, isOutput=False)
    out = nc.declare_dram_parameter("out", [NPOS, D], f32, isOutput=True)

    ntiles = NPOS // P
    W = 2 * NPOS  # sin lane cols [0:NPOS), cos lane cols [NPOS:2*NPOS)

    with TileContext(nc) as tc:
        with (
            tc.tile_pool(name="sb", bufs=1) as sb,
            tc.tile_pool(name="ps", bufs=1, space="PSUM") as ps,
        ):
            xct_sb = sb.tile([3, NPOS], f32)
            nc.sync.dma_start(out=xct_sb[:], in_=xct[:])
            omn_sb = sb.tile([3, P], f32)
            nc.sync.dma_start(out=omn_sb[:], in_=omn[:])
            b1_sb = sb.tile([P, D], f32)
            nc.sync.dma_start(out=b1_sb[:], in_=b1[:])
            b2_sb = sb.tile([P, D], f32)
            nc.sync.dma_start(out=b2_sb[:], in_=b2[:])
            bias_sb = sb.tile([P, D], f32)
            nc.sync.dma_start(out=bias_sb[:], in_=bias[:])

            magic_sb = sb.tile([P, W], f32)
            nc.any.memset(magic_sb[:], MAGIC)

            # v = phi/2pi; cos lane additionally +0.25 (i.e. +pi/2 angle)
            v_ps = ps.tile([P, W], f32)
            nc.tensor.matmul(
                v_ps[:, 0:NPOS], omn_sb[0:2, :], xct_sb[0:2, :],
                start=True, stop=True,
            )
            nc.tensor.matmul(
                v_ps[:, NPOS:W], omn_sb[:], xct_sb[:], start=True, stop=True
            )

            # k = round(v);  u = v - k in [-1/2, 1/2]
            k_sb = sb.tile([P, W], f32)
            nc.vector.scalar_tensor_tensor(
                k_sb[:], v_ps[:], MAGIC, magic_sb[:], ALU.add, ALU.subtract
            )
            u_sb = sb.tile([P, W], f32)
            nc.vector.scalar_tensor_tensor(
                u_sb[:], k_sb[:], -1.0, v_ps[:], ALU.mult, ALU.add
            )
            # cs[:, 0:NPOS] = sin(phi), cs[:, NPOS:] = cos(phi)
            cs_sb = sb.tile([P, W], f32)
            nc.scalar.activation(
                cs_sb[:], u_sb[:], mybir.ActivationFunctionType.Sin,
                bias=0.0, scale=TWO_PI,
            )

            for t in range(ntiles):
                out_ps = ps.tile([P, D], f32, tag="out_ps")
                nc.tensor.matmul(
                    out_ps[:], cs_sb[:, NPOS + t * P:NPOS + (t + 1) * P],
                    b1_sb[:], start=True, stop=False,
                )
                nc.tensor.matmul(
                    out_ps[:], cs_sb[:, t * P:(t + 1) * P], b2_sb[:],
                    start=False, stop=True,
                )
                o_sb = sb.tile([P, D], f32, tag="o_sb")
                nc.vector.tensor_add(o_sb[:], out_ps[:], bias_sb[:])
                nc.sync.dma_start(out=out[t * P:(t + 1) * P, :], in_=o_sb[:])

    nc.finalize()
    return nc


_CACHE = {}


def kernel(input, z0, om, S):
    from concourse.bass_utils import run_bass_kernel_spmd

    x = np.asarray(input, dtype=np.float32)
    z0 = np.asarray(z0, dtype=np.float32)
    om = np.asarray(om, dtype=np.float32)
    S = np.asarray(S, dtype=np.float32)

    # host prep in float64 (all O(D^2) or smaller)
    S64 = S.astype(np.float64)
    w = np.linalg.solve(S64, z0.astype(np.float64))  # S^-1 z0
    Xc = np.cumsum(x.astype(np.float64), axis=1).reshape(B * L, 2)

    w0, w1, w2 = w[0], w[1::2], w[2::2]
    A1 = S64[:, 1::2]  # [D, M]
    A2 = S64[:, 2::2]
    B1 = (A1 * w1 + A2 * w2).T  # [M, D]
    B2 = (A2 * w1 - A1 * w2).T
    biasrow = w0 * S64[:, 0]  # [D]

    omn = np.empty((3, P), dtype=np.float32)
    omn[0:2] = om.T / (2.0 * np.pi)
    omn[2] = 0.25
    b1 = np.ascontiguousarray(B1, dtype=np.float32)
    b2 = np.ascontiguousarray(B2, dtype=np.float32)
    bias = np.ascontiguousarray(
        np.broadcast_to(biasrow, (P, D)), dtype=np.float32
    )

    if "nc" not in _CACHE:
        _CACHE["nc"] = _build_nc()
    nc = _CACHE["nc"]

    in_maps = []
    for i in range(N_CORES):
        shard = np.empty((3, NPOS), dtype=np.float32)
        shard[0:2] = Xc[i * NPOS:(i + 1) * NPOS].T
        shard[2] = 1.0
        in_maps.append(
            {"xct": shard, "omn": omn, "b1": b1, "b2": b2, "bias": bias}
        )

    _CACHE["in_maps"] = in_maps
    res = run_bass_kernel_spmd(nc, in_maps, list(range(N_CORES)))
    full = np.concatenate(
        [res.results[i]["out"] for i in range(N_CORES)], axis=0
    )
    outputs = full.reshape(B, L, D).astype(np.float32)
    z_final = np.ascontiguousarray(outputs[:, -1, :])
    return outputs, z_final
